# revision 9
# baseline (speedup 1.0000x reference)
"""Trainium2 Bass kernel for a dense transformer block (nn_Block_76785425318629).

Full inputs in, full outputs out. Sharding: 8 cores = 2 batches x 4 token
quarters. Each core recomputes LN1/K/V for its batch's full sequence (avoids
all cross-core communication), and computes Q/attention/proj/MLP for its own
512 tokens. Activations flow feature-major ([C, T]) so every weight matmul is
a natural lhsT.T @ rhs contraction over partitions. LayerNorm statistics are
cross-partition sums done with ones-vector matmuls; per-token stats are
broadcast back across partitions with K=1 matmuls. Softmax skips the
max-subtraction (scores are O(1) for this problem) and applies the causal
mask multiplicatively after exp; the denominator comes from a ones-augmented
V column. Weights stream from HBM in <=8KB/partition chunks. Long-lived
activation tiles live in phase-scoped pools alternating between the left and
right SBUF heap sides so overlapping lifetimes stay legal (pool release is
LIFO per side).
"""

import sys
from contextlib import ExitStack

for _p in ("/opt/trn_rl_repo",):
    if _p not in sys.path:
        sys.path.insert(0, _p)

import numpy as np
import ml_dtypes

import concourse.bass as bass
import concourse.mybir as mybir
import concourse.tile as tile
from concourse import bacc
from concourse.bass_utils import run_bass_kernel_spmd

F32 = mybir.dt.float32
BF16 = mybir.dt.bfloat16
AF = mybir.ActivationFunctionType
OP = mybir.AluOpType

P = 128
C = 1024          # n_embd
T = 2048          # seq len
B = 2             # batch
OWN = 512         # tokens owned per core
H = 16            # heads
D = 64            # head dim
FC = 4096         # mlp hidden
KC = C // P       # 8   k-tiles over C
KF = FC // P      # 32  k-tiles over FC
KT = T // P       # 16  128-wide key tiles over T
EPS = 1e-5
N_CORES = 8
CH = 512          # column chunk
GELU_NATIVE = True  # False: tanh-approx composition (CoreSim lacks Gelu LUT)


def _layernorm_fm(nc, ps_pool, rows, rows2, xbf_pool, sq_pool, ones_col,
                  x_f32_at, h_out_at, n_cols, lw, lb):
    """Feature-major layernorm over n_cols tokens in 512-wide chunks.
    x_f32_at(kt, cs) returns a [128, CH] f32 AP (may issue a DMA);
    h_out_at(kt) returns the [128, n_cols] bf16 output tile to slice."""
    for c0 in range(0, n_cols, CH):
        cs = slice(c0, c0 + CH)
        xbf = []
        s_ps = ps_pool.tile([1, CH], F32, tag="ps")
        s2_ps = ps_pool.tile([1, CH], F32, tag="ps")
        for kt in range(KC):
            xb = xbf_pool.tile([128, CH], BF16, tag="xbf")
            nc.vector.tensor_copy(xb[:], x_f32_at(kt, cs))
            xbf.append(xb)
            sqt = sq_pool.tile([128, CH], BF16, tag="sqw")
            nc.vector.tensor_mul(sqt[:], xb[:], xb[:])
            nc.tensor.matmul(s_ps[:], ones_col[:, 0:1], xb[:],
                             start=(kt == 0), stop=(kt == KC - 1))
            nc.tensor.matmul(s2_ps[:], ones_col[:, 1:2], sqt[:],
                             start=(kt == 0), stop=(kt == KC - 1))
        mu = rows.tile([1, CH], F32, tag="row")
        var = rows.tile([1, CH], F32, tag="row")
        a_row = rows.tile([1, CH], F32, tag="row")
        b_row = rows.tile([1, CH], F32, tag="row")
        nc.vector.tensor_scalar_mul(mu[:], s_ps[:], 1.0 / C)
        nc.vector.tensor_scalar_mul(var[:], s2_ps[:], 1.0 / C)
        nc.vector.tensor_mul(b_row[:], mu[:], mu[:])
        nc.vector.tensor_sub(var[:], var[:], b_row[:])
        nc.vector.tensor_scalar_add(var[:], var[:], EPS)
        nc.scalar.activation(var[:], var[:], AF.Sqrt)
        nc.vector.reciprocal(a_row[:], var[:])
        nc.vector.tensor_mul(b_row[:], mu[:], a_row[:])
        nc.vector.tensor_scalar_mul(b_row[:], b_row[:], -1.0)
        a_bf = rows2.tile([1, CH], BF16, tag="rowbf")
        b_bf = rows2.tile([1, CH], BF16, tag="rowbf")
        nc.vector.tensor_copy(a_bf[:], a_row[:])
        nc.vector.tensor_copy(b_bf[:], b_row[:])
        a_bc = rows2.tile([128, CH], BF16, tag="abc")
        b_bc = rows2.tile([128, CH], BF16, tag="abc")
        for row_bf, bc in ((a_bf, a_bc), (b_bf, b_bc)):
            ps = ps_pool.tile([128, CH], F32, tag="ps")
            nc.tensor.matmul(ps[:], ones_col[0:1, 2:130], row_bf[:],
                             start=True, stop=True)
            nc.vector.tensor_copy(bc[:], ps[:])
        for kt in range(KC):
            ho = h_out_at(kt)[:, cs]
            nc.vector.tensor_mul(ho, xbf[kt][:], a_bc[:])
            nc.vector.tensor_add(ho, ho, b_bc[:])
            nc.vector.tensor_scalar(ho, ho, lw[:, kt:kt + 1], lb[:, kt:kt + 1],
                                    OP.mult, OP.add)


def build_program():
    nc = bacc.Bacc(None, target_bir_lowering=False)

    x_fm = nc.dram_tensor("x_fm", [C, T], F32, kind="ExternalInput")
    x_own = nc.dram_tensor("x_own", [C, OWN], F32, kind="ExternalInput")
    maskT = nc.dram_tensor("maskT", [T, OWN], BF16, kind="ExternalInput")
    wqT = nc.dram_tensor("wqT", [C, C], BF16, kind="ExternalInput")
    wkT = nc.dram_tensor("wkT", [C, C], BF16, kind="ExternalInput")
    wvT = nc.dram_tensor("wvT", [C, C], BF16, kind="ExternalInput")
    wpT = nc.dram_tensor("wpT", [C, C], BF16, kind="ExternalInput")
    wfT = nc.dram_tensor("wfT", [C, FC], BF16, kind="ExternalInput")
    wmT = nc.dram_tensor("wmT", [FC, C], BF16, kind="ExternalInput")
    biases = {}
    for name, n in (("qb", C), ("kb", C), ("vb", C), ("pb", C), ("fb", FC),
                    ("mb", C), ("l1w", C), ("l1b", C), ("l2w", C), ("l2b", C)):
        biases[name] = nc.dram_tensor(name, [n], F32, kind="ExternalInput")
    y_fm = nc.dram_tensor("y_fm", [C, OWN], F32, kind="ExternalOutput")

    wk_v = wkT.rearrange("(kt p) m -> p kt m", p=128)
    wq_v = wqT.rearrange("(kt p) m -> p kt m", p=128)
    wv_v = wvT.rearrange("(kt p) m -> p kt m", p=128)
    wp_v = wpT.rearrange("(kt p) m -> p kt m", p=128)
    wf_v = wfT.rearrange("(kt p) m -> p kt m", p=128)
    wm_v = wmT.rearrange("(kt p) m -> p kt m", p=128)
    xo_v = x_own.rearrange("(kt p) t -> p kt t", p=128)

    with tile.TileContext(nc) as tc, ExitStack() as top:
        const = top.enter_context(tc.tile_pool(name="const", bufs=1))
        ps_pool = top.enter_context(tc.tile_pool(name="ps", bufs=8, space="PSUM"))
        rows = top.enter_context(tc.tile_pool(name="rows", bufs=4))
        rows2 = top.enter_context(tc.tile_pool(name="rows2", bufs=2))
        arow = top.enter_context(tc.tile_pool(name="arow", bufs=2))
        work = top.enter_context(tc.tile_pool(name="work", bufs=3))
        xbf_pool = top.enter_context(tc.tile_pool(name="xbf", bufs=9))
        sq_pool = top.enter_context(tc.tile_pool(name="sq", bufs=2))
        wpool = top.enter_context(tc.tile_pool(name="wpool", bufs=2))
        ppool = top.enter_context(tc.tile_pool(name="ppool", bufs=6))

        # ---- constants ----
        ones_col = const.tile([128, 130], BF16)
        nc.vector.memset(ones_col[:], 1.0)

        def load_bias(name, ktiles):
            t = const.tile([128, ktiles], F32, tag=f"bias_{name}")
            nc.sync.dma_start(t[:], biases[name].rearrange("(m p) -> p m", p=128))
            return t

        qb = load_bias("qb", KC)
        kb = load_bias("kb", KC)
        pb = load_bias("pb", KC)
        fb = load_bias("fb", KF)
        mb = load_bias("mb", KC)
        l1w = load_bias("l1w", KC)
        l1b = load_bias("l1b", KC)
        l2w = load_bias("l2w", KC)
        l2b = load_bias("l2b", KC)

        vb_row_f = const.tile([1, C], F32, tag="vb_row_f")
        nc.sync.dma_start(vb_row_f[:], biases["vb"].rearrange("(o c) -> o c", o=1))
        vb_row = const.tile([1, C], BF16, tag="vb_row")
        nc.vector.tensor_copy(vb_row[:], vb_row_f[:])
        vb_bc = const.tile([128, C], F32, tag="vb_bc")
        for c0 in range(0, C, CH):
            ps = ps_pool.tile([128, CH], F32, tag="ps")
            nc.tensor.matmul(ps[:], ones_col[0:1, 2:130], vb_row[:, c0:c0 + CH],
                             start=True, stop=True)
            nc.vector.tensor_copy(vb_bc[:, c0:c0 + CH], ps[:])

        def x_from_dram(dram_slicer):
            def _at(kt, cs):
                xt = work.tile([128, CH], F32, tag="xin")
                nc.sync.dma_start(xt[:], dram_slicer(kt, cs))
                return xt[:]
            return _at

        # ===== Phase 1: LN1 (full seq + own slice) =====
        pA = tc.alloc_tile_pool(name="pA", bufs=1)
        h1 = pA.tile([128, KC, T], BF16)
        _layernorm_fm(nc, ps_pool, rows, rows2, xbf_pool, sq_pool, ones_col,
                      x_from_dram(lambda kt, cs: x_fm[kt * 128:(kt + 1) * 128, cs]),
                      lambda kt: h1[:, kt, :], T, l1w, l1b)
        h1o = pA.tile([128, KC, OWN], BF16)
        _layernorm_fm(nc, ps_pool, rows, rows2, xbf_pool, sq_pool, ones_col,
                      x_from_dram(lambda kt, cs: xo_v[:, kt, cs]),
                      lambda kt: h1o[:, kt, :], OWN, l1w, l1b)

        # ===== Phase 2: K, V, Q =====
        pB = tc.alloc_tile_pool(name="pB", bufs=1, side="right")
        k_fm = pB.tile([128, KC, T], BF16)
        for mt in range(KC):
            wk = wpool.tile([128, KC, CH], BF16, tag="w")
            nc.sync.dma_start(wk[:, :, :128], wk_v[:, :, mt * 128:(mt + 1) * 128])
            for tt in range(T // CH):
                ps = ps_pool.tile([128, CH], F32, tag="ps")
                for kt in range(KC):
                    nc.tensor.matmul(ps[:], wk[:, kt, :128],
                                     h1[:, kt, tt * CH:(tt + 1) * CH],
                                     start=(kt == 0), stop=(kt == KC - 1))
                nc.vector.tensor_scalar_add(k_fm[:, mt, tt * CH:(tt + 1) * CH],
                                            ps[:], kb[:, mt:mt + 1])

        v_aug = pB.tile([128, KT, H, D + 1], BF16)
        nc.vector.memset(v_aug[:, :, :, D:D + 1], 1.0)
        for nn in range(2):
            wv = wpool.tile([128, KC, CH], BF16, tag="w")
            nc.sync.dma_start(wv[:], wv_v[:, :, nn * CH:(nn + 1) * CH])
            for tt in range(KT):
                ps = ps_pool.tile([128, CH], F32, tag="ps")
                for kt in range(KC):
                    nc.tensor.matmul(ps[:], h1[:, kt, tt * 128:(tt + 1) * 128],
                                     wv[:, kt, :],
                                     start=(kt == 0), stop=(kt == KC - 1))
                for j in range(8):
                    h_idx = nn * 8 + j
                    nc.vector.tensor_add(v_aug[:, tt, h_idx, 0:D],
                                         ps[:, j * 64:(j + 1) * 64],
                                         vb_bc[:, h_idx * 64:h_idx * 64 + 64])

        q_own = pB.tile([128, KC, OWN], BF16)
        for mt in range(KC):
            wq = wpool.tile([128, KC, CH], BF16, tag="w")
            nc.sync.dma_start(wq[:, :, :128], wq_v[:, :, mt * 128:(mt + 1) * 128])
            ps = ps_pool.tile([128, CH], F32, tag="ps")
            for kt in range(KC):
                nc.tensor.matmul(ps[:], wq[:, kt, :128], h1o[:, kt, :],
                                 start=(kt == 0), stop=(kt == KC - 1))
            nc.vector.tensor_scalar(q_own[:, mt, :], ps[:],
                                    qb[:, mt:mt + 1], 1.0 / 8.0,
                                    OP.add, OP.mult)
        pA.release()  # h1, h1o dead

        # ===== Phase 3: attention =====
        pY = tc.alloc_tile_pool(name="pY", bufs=1)
        mask_sb = pY.tile([128, KT, OWN], BF16)
        nc.sync.dma_start(mask_sb[:], maskT.rearrange("(kt p) q -> p kt q", p=128))
        y_attn = pY.tile([128, KC, OWN], BF16)

        for h_idx in range(H):
            ft, po = h_idx // 2, (h_idx % 2) * 64
            av = ps_pool.tile([D + 1, CH], F32, tag="ps")
            p_tiles = []
            for kt in range(KT):
                ps = ps_pool.tile([128, CH], F32, tag="ps")
                nc.tensor.matmul(ps[:],
                                 k_fm[po:po + 64, ft, kt * 128:(kt + 1) * 128],
                                 q_own[po:po + 64, ft, :], start=True, stop=True)
                p_kt = ppool.tile([128, CH], BF16, tag="p")
                nc.scalar.activation(p_kt[:], ps[:], AF.Exp)
                nc.vector.tensor_mul(p_kt[:], p_kt[:], mask_sb[:, kt, :])
                p_tiles.append(p_kt)
                if kt >= 1:  # software-pipeline AV one key-tile behind scores
                    nc.tensor.matmul(av[:], v_aug[:, kt - 1, h_idx, :],
                                     p_tiles[kt - 1][:],
                                     start=(kt - 1 == 0), stop=False)
            nc.tensor.matmul(av[:], v_aug[:, KT - 1, h_idx, :],
                             p_tiles[KT - 1][:], start=False, stop=True)
            rd = arow.tile([1, OWN], F32, tag="rd")
            nc.vector.reciprocal(rd[:], av[D:D + 1, :])
            rd_bf = arow.tile([1, OWN], BF16, tag="rdbf")
            nc.vector.tensor_copy(rd_bf[:], rd[:])
            ps_bc = ps_pool.tile([64, CH], F32, tag="ps")
            nc.tensor.matmul(ps_bc[:], ones_col[0:1, 2:66], rd_bf[:],
                             start=True, stop=True)
            rd_sb = arow.tile([64, OWN], BF16, tag="rdsb")
            nc.vector.tensor_copy(rd_sb[:], ps_bc[:])
            nc.vector.tensor_mul(y_attn[po:po + 64, ft, :], av[0:D, :], rd_sb[:])
        pB.release()  # k_fm, v_aug, q_own dead

        # ===== Phase 4: proj + residual + LN2 =====
        pD = tc.alloc_tile_pool(name="pD", bufs=1, side="right")
        xow = pD.tile([128, KC, OWN], F32)
        nc.sync.dma_start(xow[:], xo_v[:])
        x2 = pD.tile([128, KC, OWN], F32)
        for mt in range(KC):
            wp = wpool.tile([128, KC, CH], BF16, tag="w")
            nc.sync.dma_start(wp[:, :, :128], wp_v[:, :, mt * 128:(mt + 1) * 128])
            ps = ps_pool.tile([128, CH], F32, tag="ps")
            for kt in range(KC):
                nc.tensor.matmul(ps[:], wp[:, kt, :128], y_attn[:, kt, :],
                                 start=(kt == 0), stop=(kt == KC - 1))
            nc.vector.tensor_scalar_add(ps[:], ps[:], pb[:, mt:mt + 1])
            nc.vector.tensor_add(x2[:, mt, :], ps[:], xow[:, mt, :])
        pY.release()  # mask, y_attn dead

        h2 = pD.tile([128, KC, OWN], BF16)
        _layernorm_fm(nc, ps_pool, rows, rows2, xbf_pool, sq_pool, ones_col,
                      lambda kt, cs: x2[:, kt, cs],
                      lambda kt: h2[:, kt, :], OWN, l2w, l2b)

        # ===== Phase 5: MLP =====
        h3 = pD.tile([128, KF, OWN], BF16)
        for mt in range(KF):
            wf = wpool.tile([128, KC, CH], BF16, tag="w")
            nc.sync.dma_start(wf[:, :, :128], wf_v[:, :, mt * 128:(mt + 1) * 128])
            ps = ps_pool.tile([128, CH], F32, tag="ps")
            for kt in range(KC):
                nc.tensor.matmul(ps[:], wf[:, kt, :128], h2[:, kt, :],
                                 start=(kt == 0), stop=(kt == KC - 1))
            if GELU_NATIVE:
                nc.scalar.activation(h3[:, mt, :], ps[:], AF.Gelu,
                                     bias=fb[:, mt:mt + 1])
            else:  # tanh-approx gelu for CoreSim
                t0 = work.tile([128, CH], F32, tag="gel")
                t3 = work.tile([128, CH], F32, tag="gel")
                nc.vector.tensor_scalar_add(t0[:], ps[:], fb[:, mt:mt + 1])
                nc.vector.tensor_mul(t3[:], t0[:], t0[:])
                nc.vector.tensor_mul(t3[:], t3[:], t0[:])
                nc.vector.tensor_scalar_mul(t3[:], t3[:], 0.044715)
                nc.vector.tensor_add(t3[:], t3[:], t0[:])
                nc.scalar.activation(t3[:], t3[:], AF.Tanh, scale=0.7978845608)
                nc.vector.tensor_scalar_add(t3[:], t3[:], 1.0)
                nc.vector.tensor_mul(t3[:], t3[:], t0[:])
                nc.vector.tensor_scalar_mul(h3[:, mt, :], t3[:], 0.5)

        for mt in range(KC):
            wm = wpool.tile([128, KF, 128], BF16, tag="w")
            nc.sync.dma_start(wm[:], wm_v[:, :, mt * 128:(mt + 1) * 128])
            ps = ps_pool.tile([128, CH], F32, tag="ps")
            for kt in range(KF):
                nc.tensor.matmul(ps[:], wm[:, kt, :], h3[:, kt, :],
                                 start=(kt == 0), stop=(kt == KF - 1))
            nc.vector.tensor_scalar_add(ps[:], ps[:], mb[:, mt:mt + 1])
            yt = work.tile([128, CH], F32, tag="yout")
            nc.vector.tensor_add(yt[:], ps[:], x2[:, mt, :])
            nc.sync.dma_start(y_fm[mt * 128:(mt + 1) * 128, :], yt[:])
        pD.release()

    nc.compile()
    return nc


_NC_CACHE = {}


def _get_nc():
    if "nc" not in _NC_CACHE:
        _NC_CACHE["nc"] = build_program()
    return _NC_CACHE["nc"]


def kernel(x, ln1_w, ln1_b, attn_w, attn_b, attn_proj_w, attn_proj_b,
           ln2_w, ln2_b, fc_w, fc_b, mlp_proj_w, mlp_proj_b):
    bf = ml_dtypes.bfloat16
    x = np.asarray(x, np.float32)
    attn_w = np.asarray(attn_w, np.float32)

    shared = {
        "wqT": np.ascontiguousarray(attn_w[0:C].T).astype(bf),
        "wkT": np.ascontiguousarray(attn_w[C:2 * C].T).astype(bf),
        "wvT": np.ascontiguousarray(attn_w[2 * C:3 * C].T).astype(bf),
        "wpT": np.ascontiguousarray(np.asarray(attn_proj_w, np.float32).T).astype(bf),
        "wfT": np.ascontiguousarray(np.asarray(fc_w, np.float32).T).astype(bf),
        "wmT": np.ascontiguousarray(np.asarray(mlp_proj_w, np.float32).T).astype(bf),
        "qb": np.ascontiguousarray(np.asarray(attn_b, np.float32)[0:C]),
        "kb": np.ascontiguousarray(np.asarray(attn_b, np.float32)[C:2 * C]),
        "vb": np.ascontiguousarray(np.asarray(attn_b, np.float32)[2 * C:3 * C]),
        "pb": np.asarray(attn_proj_b, np.float32),
        "fb": np.asarray(fc_b, np.float32),
        "mb": np.asarray(mlp_proj_b, np.float32),
        "l1w": np.asarray(ln1_w, np.float32),
        "l1b": np.asarray(ln1_b, np.float32),
        "l2w": np.asarray(ln2_w, np.float32),
        "l2b": np.asarray(ln2_b, np.float32),
    }

    x_fm_b = [np.ascontiguousarray(x[b].T) for b in range(B)]  # [C, T]
    kk = np.arange(T)[:, None]
    in_maps = []
    for core in range(N_CORES):
        b, s = divmod(core, 4)
        q0 = s * OWN
        jj = q0 + np.arange(OWN)[None, :]
        m = dict(shared)
        m["x_fm"] = x_fm_b[b]
        m["x_own"] = np.ascontiguousarray(x_fm_b[b][:, q0:q0 + OWN])
        m["maskT"] = (kk <= jj).astype(bf)
        in_maps.append(m)

    nc = _get_nc()
    res = run_bass_kernel_spmd(nc, in_maps, core_ids=list(range(N_CORES)))

    out = np.empty((B, T, C), np.float32)
    for core in range(N_CORES):
        b, s = divmod(core, 4)
        out[b, s * OWN:(s + 1) * OWN, :] = res.results[core]["y_fm"].T
    return out


# revision 20
# speedup vs baseline: 119.5531x; 119.5531x over previous
"""Trainium2 Bass kernel for a dense transformer block (nn_Block_76785425318629).

Full inputs in, full outputs out. Sharding: 8 cores = 2 batches x 4 token
quarters. Each core recomputes LN1/K/V for its batch's full sequence (avoids
all cross-core communication), and computes Q/attention/proj/MLP for its own
512 tokens. Activations flow feature-major ([C, T]) so every weight matmul is
a natural lhsT.T @ rhs contraction over partitions.

LN1 is never materialized: per-token scaling commutes through the weight
matmul (W @ (x*a) = (W@x)*a), so QKV consume raw bf16 x and each PSUM group
gets one extra K=2 rank-1 correction matmul with rows [-mu*a; 1/a] against
host-precomputed [W@1; bias] columns; the eviction is a single multiply by
the broadcast rstd. LN affine params are folded into adjacent weights on the
host (exact fp32 algebra). Cross-partition LN stats use ones-vector matmuls;
per-token stats broadcast back via K=1 matmuls. Softmax skips the
max-subtraction (scores are O(1) here) and applies the causal mask
multiplicatively after exp; the denominator comes from a ones-augmented V
column. Weights stream from HBM in <=8KB/partition chunks. Long-lived tiles
live in phase-scoped pools alternating between the left and right SBUF heap
sides (pool release is LIFO per side).
"""

import sys
from contextlib import ExitStack

for _p in ("/opt/trn_rl_repo",):
    if _p not in sys.path:
        sys.path.insert(0, _p)

import numpy as np
import ml_dtypes

import concourse.bass as bass
import concourse.mybir as mybir
import concourse.tile as tile
from concourse import bacc
from concourse.bass_utils import run_bass_kernel_spmd

F32 = mybir.dt.float32
BF16 = mybir.dt.bfloat16
AF = mybir.ActivationFunctionType
OP = mybir.AluOpType

P = 128
C = 1024          # n_embd
T = 2048          # seq len
B = 2             # batch
OWN = 512         # tokens owned per core
H = 16            # heads
D = 64            # head dim
FC = 4096         # mlp hidden
KC = C // P       # 8   k-tiles over C
KF = FC // P      # 32  k-tiles over FC
KT = T // P       # 16  128-wide key tiles over T
EPS = 1e-5
N_CORES = 8
CH = 512          # column chunk
AV_LAG = 3        # key-tiles AV trails scores by
GELU_NATIVE = True  # False: tanh-approx composition (CoreSim lacks Gelu LUT)
DEBUG_OUTS = False  # extra DRAM outputs for debugging


def _ln_stats(nc, ps_pool, rows, rows2, sq_pool, ones_col, x_bf_at, n_cols,
              a_bc_all, rs_all, a_col_all=None):
    """Stats-only feature-major layernorm over n_cols tokens (512-chunks).
    Writes, per chunk ci: broadcast rstd tile into a_bc_all[:, ci, :] (bf16),
    rows [-mu*rstd ; std] into rs_all[:, chunk] (bf16), and optionally
    per-128-token-tile rstd columns into a_col_all[:, tile] (f32)."""
    for ci, c0 in enumerate(range(0, n_cols, CH)):
        cs = slice(c0, c0 + CH)
        s_ps = ps_pool.tile([1, CH], F32, tag="ps")
        s2_ps = ps_pool.tile([1, CH], F32, tag="ps")
        for kt in range(KC):
            xb = x_bf_at(kt, cs)
            sqt = sq_pool.tile([128, CH], BF16, tag="sqw")
            nc.vector.tensor_mul(sqt[:], xb, xb)
            nc.tensor.matmul(s_ps[:], ones_col[:, 0:1], xb,
                             start=(kt == 0), stop=(kt == KC - 1))
            nc.tensor.matmul(s2_ps[:], ones_col[:, 1:2], sqt[:],
                             start=(kt == 0), stop=(kt == KC - 1))
        mu = rows.tile([1, CH], F32, tag="row")
        var = rows.tile([1, CH], F32, tag="row")
        a_row = rows.tile([1, CH], F32, tag="row")
        b_row = rows.tile([1, CH], F32, tag="row")
        nc.vector.tensor_scalar_mul(mu[:], s_ps[:], 1.0 / C)
        nc.vector.tensor_scalar_mul(var[:], s2_ps[:], 1.0 / C)
        nc.vector.tensor_mul(b_row[:], mu[:], mu[:])
        nc.vector.tensor_sub(var[:], var[:], b_row[:])
        nc.vector.tensor_scalar_add(var[:], var[:], EPS)
        nc.scalar.activation(var[:], var[:], AF.Sqrt)       # var <- std
        std_bf = rows2.tile([1, CH], BF16, tag="rowbf2")
        nc.vector.tensor_copy(std_bf[:], var[:])
        # DVE can't write partition 1; SBUF->SBUF DMA places the std row
        nc.gpsimd.dma_start(rs_all[1:2, cs], std_bf[:])
        nc.vector.reciprocal(a_row[:], var[:])              # a = rstd
        nc.vector.tensor_mul(b_row[:], mu[:], a_row[:])
        nc.vector.tensor_scalar_mul(rs_all[0:1, cs], b_row[:], -1.0)  # -mu*a
        a_bf = rows2.tile([1, CH], BF16, tag="rowbf")
        nc.vector.tensor_copy(a_bf[:], a_row[:])
        ps = ps_pool.tile([128, CH], F32, tag="ps")
        nc.tensor.matmul(ps[:], ones_col[0:1, 2:130], a_bf[:],
                         start=True, stop=True)
        nc.vector.tensor_copy(a_bc_all[:, ci, :], ps[:])
        if a_col_all is not None:
            for j in range(CH // 128):
                tt = ci * (CH // 128) + j
                pc = ps_pool.tile([128, 1], F32, tag="ps")
                nc.tensor.matmul(pc[:], a_bf[0:1, j * 128:(j + 1) * 128],
                                 ones_col[0:1, 0:1], start=True, stop=True)
                nc.vector.tensor_copy(a_col_all[:, tt:tt + 1], pc[:])


def _layernorm_fm(nc, ps_pool, rows, rows2, sq_pool, ones_col, x_bf_at,
                  h_out_at, n_cols):
    """Full feature-major layernorm (normalized output materialized)."""
    for c0 in range(0, n_cols, CH):
        cs = slice(c0, c0 + CH)
        s_ps = ps_pool.tile([1, CH], F32, tag="ps")
        s2_ps = ps_pool.tile([1, CH], F32, tag="ps")
        for kt in range(KC):
            xb = x_bf_at(kt, cs)
            sqt = sq_pool.tile([128, CH], BF16, tag="sqw")
            nc.vector.tensor_mul(sqt[:], xb, xb)
            nc.tensor.matmul(s_ps[:], ones_col[:, 0:1], xb,
                             start=(kt == 0), stop=(kt == KC - 1))
            nc.tensor.matmul(s2_ps[:], ones_col[:, 1:2], sqt[:],
                             start=(kt == 0), stop=(kt == KC - 1))
        mu = rows.tile([1, CH], F32, tag="row")
        var = rows.tile([1, CH], F32, tag="row")
        a_row = rows.tile([1, CH], F32, tag="row")
        b_row = rows.tile([1, CH], F32, tag="row")
        nc.vector.tensor_scalar_mul(mu[:], s_ps[:], 1.0 / C)
        nc.vector.tensor_scalar_mul(var[:], s2_ps[:], 1.0 / C)
        nc.vector.tensor_mul(b_row[:], mu[:], mu[:])
        nc.vector.tensor_sub(var[:], var[:], b_row[:])
        nc.vector.tensor_scalar_add(var[:], var[:], EPS)
        nc.scalar.activation(var[:], var[:], AF.Sqrt)
        nc.vector.reciprocal(a_row[:], var[:])
        nc.vector.tensor_mul(b_row[:], mu[:], a_row[:])
        nc.vector.tensor_scalar_mul(b_row[:], b_row[:], -1.0)
        a_bf = rows2.tile([1, CH], BF16, tag="rowbf")
        b_bf = rows2.tile([1, CH], BF16, tag="rowbf")
        nc.vector.tensor_copy(a_bf[:], a_row[:])
        nc.vector.tensor_copy(b_bf[:], b_row[:])
        a_bc = rows2.tile([128, CH], BF16, tag="abc")
        b_bc = rows2.tile([128, CH], BF16, tag="abc")
        for row_bf, bc in ((a_bf, a_bc), (b_bf, b_bc)):
            ps = ps_pool.tile([128, CH], F32, tag="ps")
            nc.tensor.matmul(ps[:], ones_col[0:1, 2:130], row_bf[:],
                             start=True, stop=True)
            nc.vector.tensor_copy(bc[:], ps[:])
        for kt in range(KC):
            ho = h_out_at(kt)[:, cs]
            nc.vector.tensor_mul(ho, x_bf_at(kt, cs), a_bc[:])
            nc.vector.tensor_add(ho, ho, b_bc[:])


def build_program():
    nc = bacc.Bacc(None, target_bir_lowering=False)

    x_bf = nc.dram_tensor("x_bf", [C, T], BF16, kind="ExternalInput")
    x_own = nc.dram_tensor("x_own", [C, OWN], F32, kind="ExternalInput")
    xo_bf = nc.dram_tensor("xo_bf", [C, OWN], BF16, kind="ExternalInput")
    maskT = nc.dram_tensor("maskT", [T, 128], BF16, kind="ExternalInput")
    wqT = nc.dram_tensor("wqT", [C, C], BF16, kind="ExternalInput")
    wkT = nc.dram_tensor("wkT", [C, C], BF16, kind="ExternalInput")
    wvT = nc.dram_tensor("wvT", [C, C], BF16, kind="ExternalInput")
    wpT = nc.dram_tensor("wpT", [C, C], BF16, kind="ExternalInput")
    wfT = nc.dram_tensor("wfT", [C, FC], BF16, kind="ExternalInput")
    wmT = nc.dram_tensor("wmT", [FC, C], BF16, kind="ExternalInput")
    auxq = nc.dram_tensor("auxq", [2, C], BF16, kind="ExternalInput")
    auxk = nc.dram_tensor("auxk", [2, C], BF16, kind="ExternalInput")
    auxv = nc.dram_tensor("auxv", [2, C], BF16, kind="ExternalInput")
    biases = {}
    for name, n in (("pb", C), ("fb", FC), ("mb", C)):
        biases[name] = nc.dram_tensor(name, [n], F32, kind="ExternalInput")
    y_fm = nc.dram_tensor("y_fm", [C, OWN], F32, kind="ExternalOutput")
    if DEBUG_OUTS:
        dbg_q = nc.dram_tensor("dbg_q", [128, KC, OWN], F32, kind="ExternalOutput")
        dbg_k = nc.dram_tensor("dbg_k", [128, KC, T], F32, kind="ExternalOutput")
        dbg_v = nc.dram_tensor("dbg_v", [128, KT, H, D + 1], F32, kind="ExternalOutput")
        dbg_y = nc.dram_tensor("dbg_y", [128, KC, OWN], F32, kind="ExternalOutput")

    wk_v = wkT.rearrange("(kt p) m -> p kt m", p=128)
    wq_v = wqT.rearrange("(kt p) m -> p kt m", p=128)
    wv_v = wvT.rearrange("(kt p) m -> p kt m", p=128)
    wp_v = wpT.rearrange("(kt p) m -> p kt m", p=128)
    wf_v = wfT.rearrange("(kt p) m -> p kt m", p=128)
    wm_v = wmT.rearrange("(kt p) m -> p kt m", p=128)
    xo_v = x_own.rearrange("(kt p) t -> p kt t", p=128)

    with tile.TileContext(nc) as tc, ExitStack() as top:
        const = top.enter_context(tc.tile_pool(name="const", bufs=1))
        ps_pool = top.enter_context(tc.tile_pool(name="ps", bufs=4, space="PSUM"))
        ps2_pool = top.enter_context(tc.tile_pool(name="ps2", bufs=2, space="PSUM"))
        rows = top.enter_context(tc.tile_pool(name="rows", bufs=4))
        rows2 = top.enter_context(tc.tile_pool(name="rows2", bufs=2))
        arow = top.enter_context(tc.tile_pool(name="arow", bufs=2))
        work = top.enter_context(tc.tile_pool(name="work", bufs=3))
        sq_pool = top.enter_context(tc.tile_pool(name="sq", bufs=2))
        wpool = top.enter_context(tc.tile_pool(name="wpool", bufs=3))
        wsm = top.enter_context(tc.tile_pool(name="wsm", bufs=5))
        ppool = top.enter_context(tc.tile_pool(name="ppool", bufs=4))

        # ---- constants ----
        ones_col = const.tile([128, 130], BF16)
        nc.vector.memset(ones_col[:], 1.0)

        def load_bias(name, ktiles):
            t = const.tile([128, ktiles], F32, tag=f"bias_{name}")
            nc.sync.dma_start(t[:], biases[name].rearrange("(m p) -> p m", p=128))
            return t

        pb = load_bias("pb", KC)
        fb = load_bias("fb", KF)
        mb = load_bias("mb", KC)

        auxq_sb = const.tile([2, KC, 128], BF16, tag="auxq")
        nc.sync.dma_start(auxq_sb[:], auxq.rearrange("r (kt m) -> r kt m", m=128))
        auxk_sb = const.tile([2, KC, 128], BF16, tag="auxk")
        nc.sync.dma_start(auxk_sb[:], auxk.rearrange("r (kt m) -> r kt m", m=128))
        auxv_sb = const.tile([2, C], BF16, tag="auxv")
        nc.sync.dma_start(auxv_sb[:], auxv[:, :])

        # ===== Phase 1: x residency + LN1 statistics =====
        pA = tc.alloc_tile_pool(name="pA", bufs=1)
        x_r = pA.tile([128, KC, T], BF16)
        xbf_v = x_bf.rearrange("(kt p) t -> p kt t", p=128)
        for tt in range(T // CH):          # chunk-major so chunk-0 stats start early
            cs = slice(tt * CH, (tt + 1) * CH)
            for kt in range(KC):
                nc.sync.dma_start(x_r[:, kt, cs], xbf_v[:, kt, cs])
        xo_r = pA.tile([128, KC, OWN], BF16)
        xob_v = xo_bf.rearrange("(kt p) t -> p kt t", p=128)
        for kt in range(KC):
            nc.sync.dma_start(xo_r[:, kt, :], xob_v[:, kt, :])

        a_bc_all = pA.tile([128, T // CH, CH], BF16)
        rs_all = pA.tile([2, T], BF16)
        a_col_all = pA.tile([128, KT], F32)
        a_bc_o = pA.tile([128, 1, CH], BF16)
        rs_o = pA.tile([2, CH], BF16)

        _ln_stats(nc, ps_pool, rows, rows2, sq_pool, ones_col,
                  lambda kt, cs: x_r[:, kt, cs], T, a_bc_all, rs_all, a_col_all)
        _ln_stats(nc, ps_pool, rows, rows2, sq_pool, ones_col,
                  lambda kt, cs: xo_r[:, kt, cs], OWN, a_bc_o, rs_o)

        # ===== Phase 2: K, V, Q =====
        pB = tc.alloc_tile_pool(name="pB", bufs=1, side="right")
        k_fm = pB.tile([128, KC, T], BF16)
        for mt in range(KC):
            wk = wsm.tile([128, KC, 128], BF16, tag="w")
            nc.sync.dma_start(wk[:], wk_v[:, :, mt * 128:(mt + 1) * 128])
            for tt in range(T // CH):
                cs = slice(tt * CH, (tt + 1) * CH)
                ps = ps_pool.tile([128, CH], F32, tag="ps")
                for kt in range(KC):
                    nc.tensor.matmul(ps[:], wk[:, kt, :], x_r[:, kt, cs],
                                     start=(kt == 0), stop=False)
                nc.tensor.matmul(ps[:], auxk_sb[:, mt, :], rs_all[:, cs],
                                 start=False, stop=True)
                nc.vector.tensor_mul(k_fm[:, mt, cs], ps[:], a_bc_all[:, tt, :])

        v_aug = pB.tile([128, KT, H, D + 1], BF16)
        nc.vector.memset(v_aug[:, :, :, D:D + 1], 1.0)
        for nn in range(2):
            ncs = slice(nn * CH, (nn + 1) * CH)
            wv = wpool.tile([128, KC, CH], BF16, tag="w")
            nc.sync.dma_start(wv[:], wv_v[:, :, ncs])
            for tt in range(KT):
                ts_ = slice(tt * 128, (tt + 1) * 128)
                ps = ps_pool.tile([128, CH], F32, tag="ps")
                for kt in range(KC):
                    nc.tensor.matmul(ps[:], x_r[:, kt, ts_], wv[:, kt, :],
                                     start=(kt == 0), stop=False)
                nc.tensor.matmul(ps[:], rs_all[:, ts_], auxv_sb[:, ncs],
                                 start=False, stop=True)
                for j in range(8):
                    h_idx = nn * 8 + j
                    nc.vector.tensor_scalar_mul(v_aug[:, tt, h_idx, 0:D],
                                                ps[:, j * 64:(j + 1) * 64],
                                                a_col_all[:, tt:tt + 1])

        q_own = pB.tile([128, KC, OWN], BF16)
        for mt in range(KC):
            wq = wsm.tile([128, KC, 128], BF16, tag="w")
            nc.sync.dma_start(wq[:], wq_v[:, :, mt * 128:(mt + 1) * 128])
            ps = ps_pool.tile([128, CH], F32, tag="ps")
            for kt in range(KC):
                nc.tensor.matmul(ps[:], wq[:, kt, :], xo_r[:, kt, :],
                                 start=(kt == 0), stop=False)
            nc.tensor.matmul(ps[:], auxq_sb[:, mt, :], rs_o[:, :],
                             start=False, stop=True)
            nc.vector.tensor_mul(q_own[:, mt, :], ps[:], a_bc_o[:, 0, :])
        if DEBUG_OUTS:
            for t, d in ((q_own, dbg_q), (k_fm, dbg_k), (v_aug, dbg_v)):
                nc.gpsimd.dma_start(d[:], t[:])
        pA.release()  # x_r, xo_r, stats dead

        # ===== Phase 3: attention =====
        pY = tc.alloc_tile_pool(name="pY", bufs=1)
        # with strided token ownership (core owns tokens s, s+4, ...), local
        # q-block i spans globals [512i, 512i+512) on every core: key-tile kt
        # is needed only by q columns >= 128*(kt//4), and only the first
        # 128-wide q-block of that range crosses the diagonal (masked from
        # per-core data); later q-blocks are fully allowed, earlier skipped.
        mask_sb = pY.tile([128, KT, 128], BF16)
        nc.sync.dma_start(mask_sb[:], maskT.rearrange("(kt p) q -> p kt q", p=128))
        y_attn = pY.tile([128, KC, OWN], BF16)

        NPAIR = KT // 2
        for h_idx in range(H):
            ft, po = h_idx // 2, (h_idx % 2) * 64
            av = ps_pool.tile([D + 1, CH], F32, tag="ps")
            p_pairs = []

            def _av_pair(pr):
                p_pair, jq0, jw = p_pairs[pr]
                for half in range(2):
                    kt = 2 * pr + half
                    nc.tensor.matmul(av[:, jq0:OWN],
                                     v_aug[:, kt, h_idx, :],
                                     p_pair[:, half, 0:jw],
                                     start=(kt == 0), stop=(kt == KT - 1),
                                     skip_group_check=True)

            for pr in range(NPAIR):
                kt0 = 2 * pr
                q0c = 128 * (kt0 // 4)         # first q column needing kt0
                w = OWN - q0c
                ps2 = ps2_pool.tile([128, 2, CH], F32, tag="ps2")
                for half in range(2):
                    kt = kt0 + half
                    nc.tensor.matmul(ps2[:, half, 0:w],
                                     k_fm[po:po + 64, ft, kt * 128:(kt + 1) * 128],
                                     q_own[po:po + 64, ft, q0c:OWN],
                                     start=True, stop=True)
                p_pair = ppool.tile([128, 2, CH], BF16, tag="p")
                nc.scalar.activation(p_pair[:, :, 0:w], ps2[:, :, 0:w], AF.Exp)
                nc.vector.tensor_mul(p_pair[:, :, 0:128], p_pair[:, :, 0:128],
                                     mask_sb[:, kt0:kt0 + 2, :])
                p_pairs.append((p_pair, q0c, w))
                if pr >= 2:        # AV lags scores by 2 pairs
                    _av_pair(pr - 2)
            for pr in range(NPAIR - 2, NPAIR):
                _av_pair(pr)
            rd = arow.tile([1, OWN], F32, tag="rd")
            nc.vector.reciprocal(rd[:], av[D:D + 1, :])
            rd_bf = arow.tile([1, OWN], BF16, tag="rdbf")
            nc.vector.tensor_copy(rd_bf[:], rd[:])
            ps_bc = ps_pool.tile([64, CH], F32, tag="ps")
            nc.tensor.matmul(ps_bc[:], ones_col[0:1, 2:66], rd_bf[:],
                             start=True, stop=True)
            rd_sb = arow.tile([64, OWN], BF16, tag="rdsb")
            nc.vector.tensor_copy(rd_sb[:], ps_bc[:])
            nc.vector.tensor_mul(y_attn[po:po + 64, ft, :], av[0:D, :], rd_sb[:])
        if DEBUG_OUTS:
            nc.gpsimd.dma_start(dbg_y[:], y_attn[:])
        pB.release()  # k_fm, v_aug, q_own dead

        # ===== Phase 4: proj + residual + LN2 =====
        pD = tc.alloc_tile_pool(name="pD", bufs=1, side="right")
        xow = pD.tile([128, KC, OWN], F32)
        nc.sync.dma_start(xow[:], xo_v[:])
        x2 = pD.tile([128, KC, OWN], F32)
        for mt in range(KC):
            wp = wsm.tile([128, KC, 128], BF16, tag="w")
            nc.sync.dma_start(wp[:], wp_v[:, :, mt * 128:(mt + 1) * 128])
            ps = ps_pool.tile([128, CH], F32, tag="ps")
            for kt in range(KC):
                nc.tensor.matmul(ps[:], wp[:, kt, :], y_attn[:, kt, :],
                                 start=(kt == 0), stop=(kt == KC - 1))
            nc.vector.tensor_scalar_add(ps[:], ps[:], pb[:, mt:mt + 1])
            nc.vector.tensor_add(x2[:, mt, :], ps[:], xow[:, mt, :])
        pY.release()  # mask, y_attn dead

        x2_bf = pD.tile([128, KC, OWN], BF16)
        for kt in range(KC):
            nc.vector.tensor_copy(x2_bf[:, kt, :], x2[:, kt, :])
        h2 = pD.tile([128, KC, OWN], BF16)
        _layernorm_fm(nc, ps_pool, rows, rows2, sq_pool, ones_col,
                      lambda kt, cs: x2_bf[:, kt, cs],
                      lambda kt: h2[:, kt, :], OWN)

        # ===== Phase 5: MLP =====
        h3 = pD.tile([128, KF, OWN], BF16)
        for mt in range(KF):
            wf = wsm.tile([128, KC, 128], BF16, tag="w")
            nc.sync.dma_start(wf[:], wf_v[:, :, mt * 128:(mt + 1) * 128])
            ps = ps_pool.tile([128, CH], F32, tag="ps")
            for kt in range(KC):
                nc.tensor.matmul(ps[:], wf[:, kt, :], h2[:, kt, :],
                                 start=(kt == 0), stop=(kt == KC - 1))
            if GELU_NATIVE:
                nc.scalar.activation(h3[:, mt, :], ps[:], AF.Gelu,
                                     bias=fb[:, mt:mt + 1])
            else:  # tanh-approx gelu for CoreSim
                t0 = work.tile([128, CH], F32, tag="gel")
                t3 = work.tile([128, CH], F32, tag="gel")
                nc.vector.tensor_scalar_add(t0[:], ps[:], fb[:, mt:mt + 1])
                nc.vector.tensor_mul(t3[:], t0[:], t0[:])
                nc.vector.tensor_mul(t3[:], t3[:], t0[:])
                nc.vector.tensor_scalar_mul(t3[:], t3[:], 0.044715)
                nc.vector.tensor_add(t3[:], t3[:], t0[:])
                nc.scalar.activation(t3[:], t3[:], AF.Tanh, scale=0.7978845608)
                nc.vector.tensor_scalar_add(t3[:], t3[:], 1.0)
                nc.vector.tensor_mul(t3[:], t3[:], t0[:])
                nc.vector.tensor_scalar_mul(h3[:, mt, :], t3[:], 0.5)

        for mt in range(KC):
            wm = wpool.tile([128, KF, 128], BF16, tag="w")
            nc.sync.dma_start(wm[:], wm_v[:, :, mt * 128:(mt + 1) * 128])
            ps = ps_pool.tile([128, CH], F32, tag="ps")
            for kt in range(KF):
                nc.tensor.matmul(ps[:], wm[:, kt, :], h3[:, kt, :],
                                 start=(kt == 0), stop=(kt == KF - 1))
            nc.vector.tensor_scalar_add(ps[:], ps[:], mb[:, mt:mt + 1])
            yt = work.tile([128, CH], F32, tag="yout")
            nc.vector.tensor_add(yt[:], ps[:], x2[:, mt, :])
            nc.sync.dma_start(y_fm[mt * 128:(mt + 1) * 128, :], yt[:])
        pD.release()

    nc.compile()
    return nc


_NC_CACHE = {}


def _get_nc():
    if "nc" not in _NC_CACHE:
        _NC_CACHE["nc"] = build_program()
    return _NC_CACHE["nc"]


def make_in_maps(x, ln1_w, ln1_b, attn_w, attn_b, attn_proj_w, attn_proj_b,
                 ln2_w, ln2_b, fc_w, fc_b, mlp_proj_w, mlp_proj_b):
    """Host-side marshalling: feature-major layouts, bf16 casts, LN affine
    folded into the adjacent weight matrices (exact fp32 algebra), Q scaled
    by 1/sqrt(D), rank-1 LN-correction aux rows [W@1; bias]."""
    bf = ml_dtypes.bfloat16
    f32 = np.float32
    x = np.asarray(x, f32)
    attn_w = np.asarray(attn_w, f32)
    attn_b = np.asarray(attn_b, f32)
    ln1_w = np.asarray(ln1_w, f32); ln1_b = np.asarray(ln1_b, f32)
    ln2_w = np.asarray(ln2_w, f32); ln2_b = np.asarray(ln2_b, f32)
    fc_w = np.asarray(fc_w, f32); fc_b = np.asarray(fc_b, f32)

    wq, wk, wv = attn_w[0:C], attn_w[C:2 * C], attn_w[2 * C:3 * C]
    bq, bk, bv = attn_b[0:C], attn_b[C:2 * C], attn_b[2 * C:3 * C]
    wq_e = (wq * ln1_w[None, :]) * 0.125
    bq_e = (bq + wq @ ln1_b) * 0.125
    wk_e = wk * ln1_w[None, :]
    bk_e = bk + wk @ ln1_b
    wv_e = wv * ln1_w[None, :]
    bv_e = bv + wv @ ln1_b
    fc_e = fc_w * ln2_w[None, :]
    fb_e = fc_b + fc_w @ ln2_b

    shared = {
        "wqT": np.ascontiguousarray(wq_e.T).astype(bf),
        "wkT": np.ascontiguousarray(wk_e.T).astype(bf),
        "wvT": np.ascontiguousarray(wv_e.T).astype(bf),
        "wpT": np.ascontiguousarray(np.asarray(attn_proj_w, f32).T).astype(bf),
        "wfT": np.ascontiguousarray(fc_e.T).astype(bf),
        "wmT": np.ascontiguousarray(np.asarray(mlp_proj_w, f32).T).astype(bf),
        "auxq": np.ascontiguousarray(np.stack([wq_e.sum(1), bq_e])).astype(bf),
        "auxk": np.ascontiguousarray(np.stack([wk_e.sum(1), bk_e])).astype(bf),
        "auxv": np.ascontiguousarray(np.stack([wv_e.sum(1), bv_e])).astype(bf),
        "pb": np.asarray(attn_proj_b, f32),
        "fb": np.ascontiguousarray(fb_e),
        "mb": np.asarray(mlp_proj_b, f32),
    }

    x_fm_b = [np.ascontiguousarray(x[b].T) for b in range(B)]  # [C, T]
    x_bf_b = [xb.astype(bf) for xb in x_fm_b]
    in_maps = []
    for core in range(N_CORES):
        b, s = divmod(core, 4)
        # strided ownership: local query j <-> global token 4j + s
        m = dict(shared)
        m["x_bf"] = x_bf_b[b]
        m["x_own"] = np.ascontiguousarray(x_fm_b[b][:, s::4])
        m["xo_bf"] = np.ascontiguousarray(x_bf_b[b][:, s::4])
        # diagonal-band mask: for key-tile kt, q-block i = kt//4 (local q
        # columns 128i..128i+127 = local queries q0c+jp): allowed iff
        # key_glob <= 4*(128*(kt//4) + jp) + s  with key_glob = 128*kt + kr
        kr = np.arange(T)[:, None]          # global key index
        jp = np.arange(128)[None, :]        # position within the q-block
        ktile = kr // 128
        qglob = 512 * (ktile // 4) + 4 * jp + s
        m["maskT"] = (kr <= qglob).astype(bf)
        in_maps.append(m)
    return in_maps


def assemble_output(results):
    out = np.empty((B, T, C), np.float32)
    for core in range(N_CORES):
        b, s = divmod(core, 4)
        out[b, s::4, :] = results[core]["y_fm"].T
    return out


def kernel(**inputs):
    in_maps = make_in_maps(**inputs)
    nc = _get_nc()
    res = run_bass_kernel_spmd(nc, in_maps, core_ids=list(range(N_CORES)))
    return assemble_output(res.results)
